# revision 49
# baseline (speedup 1.0000x reference)
"""Trainium2 Bass kernel for nn_Block_13391708030014 (dense transformer block).

Sharding: data-parallel over batch — core b computes batch item b entirely
(B == n_cores == 8), no collectives.

The target runtime dispatches instructions with a large fixed per-instruction
cost (engines effectively serialized), so the design minimizes TOTAL
instruction count (~1.1k bass / ~1.4k NEFF vs 2.0k/2.4k for the previous
iteration, which itself was down from ~7.6k). Matmul count (520) sits at the
PSUM-output/contraction floor for this decomposition; everything else is
batched into as few giant instructions as the ISA allows:

  A. x loaded in ONE DMA; ln1 stats for ALL 32 token tiles in ~9 ops
     (Square on ACT + two inner-axis tensor_reduce + small fixups);
     32 per-tile normalizes (per-partition scalar limit); ONE batched
     xbar DMA-transpose for all of h ([128, 16KB] -> 64 blocks) + ONE
     fp8 cast; q as 16 DR matmuls under 2 ldweights (8 PSUM banks each,
     single whole-PSUM evictions).
  B. SR conv: 16 taps x 2 c-chunks as 32 DR matmuls on strided views of
     h^T(fp8); srn stats via reduce; one batched transpose each way; the
     block-diagonal v8bd/va8bd/onesbd stationaries built with ONE strided
     copy each (hh AP step = block_stride+32); (1-alpha) folded into vw.
  C. per 2048-token quad: pos loaded in ONE DMA and pos@va computed first
     (4 ldweights per head-group after dedup, 8 accumulator banks, ONE
     eviction per quad); scores TRANSPOSED (sT[nk,t] = k_h^T q_h): the
     PE's four 32-row strips hold the 4 heads' k stationaries
     INDEPENDENTLY (tile_position row groups - a 32-row ldweights only
     clobbers its own strip), so per nk-block the 4 stationaries load
     once and feed all 4 token chunks (8 ldweights + 4 Exps per
     (quad, head-group)); G and attn@v as DR matmuls with block-diagonal
     stationaries merged across BOTH head-groups per quad (shared onesbd
     ldweights, 8-bank accumulators, ONE reciprocal + two tensor ops per
     quad). proj computed TRANSPOSED (pT[c,t], 8 matmuls per c-block
     under 1 ldweights), one eviction + one batched transpose + one
     residual add per c-block; ln2 like phase A.
  D. fc1: per hidden block ONE ldweights + 8 DR matmuls into all 8 PSUM
     banks, ONE eviction into the zero-padded 66x66 spatial layout;
     depthwise 3x3 conv as 9 scalar_tensor_tensor chains on DVE;
     bias+Gelu fused into one fp8 eviction per block; fc2 TRANSPOSED
     with stationary-outer loops (4 ldweights per c-block feed 32
     matmuls into 8 banks), ONE eviction/transpose/residual-add per
     c-block; output stored in ONE DMA.

Cross-cutting passes (in _run): matmul waits folded onto ldweights
(bass_rust); consecutive same-stationary InstLdweights removed (PE array
keeps weights across matmuls - verified on HW), their waits reattached to
the following matmul; same-semaphore waits merged to the max value;
remaining multi-wait instructions split onto 2-wait EventSemaphore NOPs
(walrus 1-wait limit). HWDGE DMA completion collapsed to one sem lane.
Stats/x tiles stride-padded so the AP optimizer cannot merge token groups.
Hardware rel err ~7.4e-3 (fp8 noise; tolerance 2e-2).
"""

from contextlib import ExitStack

import numpy as np
import ml_dtypes

import concourse.bass as bass
import concourse.tile as tile
from concourse import mybir
from concourse.bass_utils import run_bass_kernel_spmd

F32 = mybir.dt.float32
BF16 = mybir.dt.bfloat16
FP8 = mybir.dt.float8e4
AF = mybir.ActivationFunctionType
OP = mybir.AluOpType
DR = mybir.MatmulPerfMode.DoubleRow

B, N, C = 8, 4096, 256
H, DH = 8, 32
NK = 256
HID = 1024
HW = 64
SR = 4
P = 128
TT = N // P          # 32 token tiles
KB = C // P          # 2 channel blocks
MB = HID // P        # 8 hidden blocks
PADW = HW + 2        # 66
CP = C + 1           # stride-padded token row (prevents AP dim-merge)
NPAD = PADW * PADW   # 4356


def _split_drain_waits(nc, max_waits=1, dma_only=False):
    """walrus refuses >1 sem wait per instruction (2 on InstEventSemaphore).
    Keep the first wait on the instruction and hoist the rest, packed in
    pairs, onto InstEventSemaphore instructions inserted just before it on
    the same engine (semantically identical: same engine, program order).
    dma_only=True limits splitting to DMA-ish instructions (experiment:
    walrus appears to lower compute-instruction waits as standalone
    SEMAPHORE ops anyway)."""
    dma_types = ("InstDMACopy", "InstDmaTransposeAnt", "InstDrain",
                 "InstTensorLoad", "InstTensorSave")
    for f in nc.m.functions:
        for blk in f.blocks:
            insts = blk.instructions
            new = []
            changed = False
            for inst in insts:
                si = getattr(inst, "sync_info", None)
                if dma_only and type(inst).__name__ not in dma_types:
                    new.append(inst)
                    continue
                if si is not None and si.on_wait and len(si.on_wait) > max_waits:
                    waits = list(si.on_wait)
                    extra = waits[max_waits:]
                    for i in range(0, len(extra), 2):
                        new.append(mybir.InstEventSemaphore(
                            name=f"{inst.name}-ws{i}",
                            sync_info=mybir.SyncInfo(
                                on_wait=extra[i:i + 2], on_update=[]),
                            bass_nofuse=True,
                            engine=inst.engine,
                            ins=[], outs=[],
                        ))
                    inst.sync_info = mybir.SyncInfo(
                        on_wait=waits[:max_waits],
                        on_update=list(si.on_update or []))
                    changed = True
                new.append(inst)
            if changed:
                blk.instructions = new


def _move_matmul_waits(nc):
    """Fold matmul waits onto the paired ldweights (no extra instructions)."""
    try:
        import bass_rust
        bass_rust.move_matmul_waits_to_ldweights(nc.m)
    except Exception:
        pass


def _merge_waits(nc):
    """Merge sem-ge-imm waits on the same semaphore: keep the max value.
    (Waits are monotone >= conditions, so the max implies the rest.)"""
    for f in nc.m.functions:
        for blk in f.blocks:
            for inst in blk.instructions:
                si = getattr(inst, "sync_info", None)
                if si is None or not si.on_wait or len(si.on_wait) < 2:
                    continue
                best, order, rest = {}, [], []
                for w in si.on_wait:
                    if (w.sync_type == "semaphore"
                            and w.wait_mode == "sem-ge-imm"
                            and w.wait_reg is None):
                        if w.id not in best:
                            best[w.id] = w
                            order.append(w.id)
                        elif w.wait_value > best[w.id].wait_value:
                            best[w.id] = w
                    else:
                        rest.append(w)
                merged = [best[k] for k in order] + rest
                if len(merged) < len(si.on_wait):
                    inst.sync_info = mybir.SyncInfo(
                        on_wait=merged, on_update=list(si.on_update or []))


# SBUF tiles that are written once (before any dependent ldweights) and then
# only read: safe targets for ldweights dedup.
_LDW_STABLE_PREFIXES = (
    "qw_sb", "srw_sb", "kw_sb", "vw_sb", "pjw_sb", "f1w_sb", "f2w_sb",
    "kT", "v8bd", "va8bd", "onesbd",
)


def _dedup_ldweights(nc):
    """Remove an InstLdweights when the immediately preceding PE ldweights
    loaded the identical stationary (same AP/perf_mode/tile_position) and the
    tile is write-once (whitelist). The PE array keeps weights across matmuls,
    so the duplicate load is redundant. Any waits on the removed instruction
    move to the next PE instruction (its matmul) - program order on the PE
    engine is unchanged, so semantics are preserved."""
    PE = mybir.EngineType.PE
    n_removed = 0
    for f in nc.m.functions:
        for blk in f.blocks:
            insts = blk.instructions
            # indices of PE instructions in stream order
            pe_idx = [i for i, inst in enumerate(insts)
                      if getattr(inst, "engine", None) == PE]
            drop = set()
            pending_waits = {}  # target stream index -> list of waits
            # The 128x128 PE array is 4 independent 32-row strips
            # (tile_position row groups); a 32-row ldweights only
            # clobbers its own strip, so track the resident stationary
            # per strip.
            strip_key = [None] * 4
            for j, i in enumerate(pe_idx):
                inst = insts[i]
                nm = type(inst).__name__
                if nm == "InstLdweights":
                    key = (repr(inst.ins), repr(inst.perf_mode),
                           repr(inst.tile_position),
                           repr(getattr(inst, "is_transpose", None)))
                    tp = getattr(inst, "tile_position", None)
                    ts_ = getattr(inst, "tile_size", None)
                    r0 = tp[0] if tp else 0
                    nr = ts_[0] if ts_ else 128
                    strips = range(r0 // 32, min(4, (r0 + nr + 31) // 32))
                    stable = any(p in repr(inst.ins)
                                 for p in _LDW_STABLE_PREFIXES)
                    if (stable and j + 1 < len(pe_idx)
                            and all(strip_key[s] == key for s in strips)):
                        si = getattr(inst, "sync_info", None)
                        if si is not None and si.on_wait:
                            tgt = pe_idx[j + 1]
                            pending_waits.setdefault(tgt, []).extend(
                                si.on_wait)
                        drop.add(i)
                        n_removed += 1
                        continue
                    for s in strips:
                        strip_key[s] = key
                elif nm == "InstMatmult":
                    pass  # does not clobber loaded weights
                elif nm in ("InstEventSemaphore", "InstDrain", "InstNop"):
                    pass  # no effect on the PE array
                else:
                    strip_key = [None] * 4  # unknown PE instr: be safe
            if not drop:
                continue
            for tgt, waits in pending_waits.items():
                inst = insts[tgt]
                si = getattr(inst, "sync_info", None)
                old = list(si.on_wait) if si is not None and si.on_wait else []
                upd = list(si.on_update or []) if si is not None else []
                inst.sync_info = mybir.SyncInfo(on_wait=old + waits,
                                                on_update=upd)
            blk.instructions = [inst for i, inst in enumerate(insts)
                                if i not in drop]
    return n_removed


def _bf(x):
    return np.ascontiguousarray(x.astype(ml_dtypes.bfloat16))


def _f8(x):
    return np.ascontiguousarray(x.astype(ml_dtypes.float8_e4m3))


def _prep_weights(i, a):
    """Fold LN affines + (1-alpha) into weights; return DRAM payloads."""
    ln1_w, ln1_b = i["ln1_w"], i["ln1_b"]
    ln2_w, ln2_b = i["ln2_w"], i["ln2_b"]

    qw = ln1_w[:, None] * i["q_w"]                      # [C, C]
    qb = ln1_b @ i["q_w"] + i["q_b"]                    # [C]

    # sr_w is OIHW: [c_out, c_in, dy, dx] -> srw[tap, ci, co]
    srw = (i["sr_w"] * ln1_w[None, :, None, None]).transpose(2, 3, 1, 0)
    srw = np.ascontiguousarray(srw.reshape(SR * SR, C, C))
    srb = i["sr_b"] + np.einsum("i,oihw->o", ln1_b, i["sr_w"])

    srn_w, srn_b = i["srn_w"], i["srn_b"]
    kvw = srn_w[:, None] * i["kv_w"]                    # [C, 2C]
    kvb = srn_b @ i["kv_w"] + i["kv_b"]
    kw, vw = kvw[:, :C], kvw[:, C:]
    kb_, vb = kvb[:C], kvb[C:]
    # fold (1-a) into the v weights (the softmax path); the pos path then
    # multiplies by a/(1-a) to recover alpha*v.
    vw1 = (1.0 - a) * vw
    vb1 = (1.0 - a) * vb

    f1w = ln2_w[:, None] * i["fc1_w"]                   # [C, HID]
    f1b = ln2_b @ i["fc1_w"] + i["fc1_b"]

    dww = i["dw_w"].reshape(HID, 9)                     # [HID, tap]
    # [128, MB, 9] per-partition scalars
    dww_p = np.ascontiguousarray(
        dww.reshape(MB, P, 9).transpose(1, 0, 2))

    # fc2 as fp8 DoubleRow over hidden-block pairs: [4, 128, 2, C]
    f2w8 = np.ascontiguousarray(
        i["fc2_w"].reshape(MB // 2, 2, P, C).transpose(0, 2, 1, 3))

    # proj fp8 DoubleRow over c-chunk pairs: [128, 2, C]
    pjw8 = np.ascontiguousarray(
        i["proj_w"].reshape(KB, P, C).transpose(1, 0, 2))

    # fp8 DoubleRow layouts pairing the two c-in chunks: [128, 2, out]
    qw8 = np.ascontiguousarray(qw.reshape(KB, P, C).transpose(1, 0, 2))
    srw8 = np.ascontiguousarray(
        srw.reshape(16, KB, P, C).transpose(2, 0, 1, 3))  # [128, 16, 2, C]
    f1w8 = np.ascontiguousarray(f1w.reshape(KB, P, HID).transpose(1, 0, 2))

    return {
        "qw8": _f8(qw8), "qb": qb.astype(np.float32),
        "srw8": _f8(srw8), "srb": srb.astype(np.float32),
        "kw": _bf(kw), "kb": kb_.astype(np.float32),
        "vw": _bf(vw1), "vb": vb1.astype(np.float32),
        "pjw8": _f8(pjw8), "pjb": i["proj_b"].astype(np.float32),
        "f1w8": _f8(f1w8), "f1b": f1b.astype(np.float32),
        "dww": dww_p.astype(np.float32),
        "dwb": i["dw_b"].astype(np.float32),
        "f2w8": _f8(f2w8), "f2b": i["fc2_b"].astype(np.float32),
    }


def _build_program(a, nz):
    # Collapse HWDGE DMA completion tracking to one sem lane: all DMAs issue
    # from SP (one FIFO ring), so cumulative single-sem waits are safe, and
    # consumers of multi-DMA regions then need 1 wait instead of up to 8
    # (the target runtime charges a fixed ~5us per instruction, and every
    # extra wait becomes an extra instruction).
    import concourse.tile_sem_assignment as _tsa
    _saved_sems = _tsa.NUM_HWDGE_SEMS
    _tsa.NUM_HWDGE_SEMS = 1
    try:
        return _build_program_inner(a, nz)
    finally:
        _tsa.NUM_HWDGE_SEMS = _saved_sems


def _stats_via_reduce(nc, pool, src_ap, eps_tile, tag):
    """Batched LN stats: per-group mean + inv-std over the innermost free
    dim(s) of src_ap [128, G, inner...] in ~9 instructions regardless of G.
    Returns the stats tile; mean at [:, 2, g], inv-std at [:, 5, g]."""
    shp = src_ap.shape
    G = shp[1]
    inner = list(shp[2:])
    nelem = 1
    for d in inner:
        nelem *= d
    axis = mybir.AxisListType.X if len(inner) == 1 else mybir.AxisListType.XY
    sq = pool.tile([P, G, nelem + 8], BF16, tag=f"sq_{tag}",
                   name=f"sq_{tag}", bufs=1)
    sqv = sq[:, :, 0:nelem]
    if len(inner) == 2:
        sqv = sqv.rearrange("p g (a b) -> p g a b", a=inner[0])
    nc.scalar.activation(sqv, src_ap, AF.Square)
    st = pool.tile([P, 6, G + 1], F32, tag=f"st_{tag}", name=f"st_{tag}",
                   bufs=1)
    nc.vector.tensor_reduce(out=st[:, 0, 0:G], in_=src_ap, axis=axis,
                            op=OP.add)
    nc.vector.tensor_reduce(out=st[:, 1, 0:G], in_=sqv, axis=axis, op=OP.add)
    nc.vector.tensor_scalar(out=st[:, 2, 0:G], in0=st[:, 0, 0:G],
                            scalar1=1.0 / nelem, scalar2=None, op0=OP.mult)
    nc.vector.tensor_tensor(out=st[:, 3, 0:G], in0=st[:, 2, 0:G],
                            in1=st[:, 2, 0:G], op=OP.mult)
    # var = s2/nelem - mu^2 in one scalar_tensor_tensor
    nc.vector.scalar_tensor_tensor(
        out=st[:, 4, 0:G], in0=st[:, 1, 0:G], scalar=1.0 / nelem,
        in1=st[:, 3, 0:G], op0=OP.mult, op1=OP.subtract)
    nc.scalar.activation(st[:, 5, 0:G], st[:, 4, 0:G], AF.Sqrt,
                         bias=eps_tile[:])
    nc.vector.reciprocal(st[:, 5, 0:G], st[:, 5, 0:G])
    return st


def _build_program_inner(a, nz):
    nc = bass.Bass("TRN2", target_bir_lowering=False, debug=False,
                   num_devices=B)

    x_d = nc.dram_tensor("x", [N, C], F32, kind="ExternalInput").ap()
    # pos, host-packed to [p(nk%128), hg, nkb, hh, N] fp8
    pos_d = nc.dram_tensor("pos8", [P, KB, KB, 4, N], FP8,
                           kind="ExternalInput").ap()
    out_d = nc.dram_tensor("out", [N, C], F32, kind="ExternalOutput").ap()

    w_d = {}
    wshapes = {
        "qw8": ([P, KB, C], FP8), "srw8": ([P, 16, KB, C], FP8),
        "kw": ([C, C], BF16), "vw": ([C, C], BF16),
        "pjw8": ([P, KB, C], FP8), "f1w8": ([P, KB, HID], FP8),
        "dww": ([P, MB, 9], F32), "dwb": ([HID], F32),
        "f2w8": ([MB // 2, P, 2, C], FP8),
    }
    for nm in ("qb", "srb", "kb", "vb", "pjb", "f1b", "f2b"):
        if nz[nm]:
            wshapes[nm] = ([{"f1b": HID}.get(nm, C)], F32)
    for nm, (shp, dt) in wshapes.items():
        w_d[nm] = nc.dram_tensor(nm, shp, dt, kind="ExternalInput").ap()

    scale = DH ** -0.5
    av_s = a / (1.0 - a) if abs(1.0 - a) > 1e-12 else 0.0

    with tile.TileContext(nc) as tc, ExitStack() as ctx:
        persist = ctx.enter_context(tc.tile_pool(name="persist", bufs=1))
        wpool = ctx.enter_context(tc.tile_pool(name="weights", bufs=1))
        stat = ctx.enter_context(tc.tile_pool(name="stat", bufs=4))

        # ---- persistent tiles
        hcT8 = persist.tile([P, KB, N], FP8, tag="hcT8")
        qT = persist.tile([P, KB, N], BF16, tag="qT")
        kT = persist.tile([P, KB, NK], BF16, tag="kT")
        v8 = persist.tile([P, KB, C], FP8, tag="v8")
        v8bd = persist.tile([P, KB, 4, KB, P], FP8, tag="v8bd")
        onesbd = persist.tile([P, 4, KB, P], FP8, tag="onesbd")
        xfull = persist.tile([P, TT, CP], F32, tag="xfull")
        x2 = persist.tile([P, TT, CP], F32, tag="x2")
        h2T8 = persist.tile([P, KB, N], FP8, tag="h2T8")

        eps1 = persist.tile([P, 1], F32, tag="eps1")
        nc.vector.memset(eps1[:], 1e-6)
        epss = persist.tile([P, 1], F32, tag="epss")
        nc.vector.memset(epss[:], 1e-5)

        # ---- weights to SBUF
        qw_sb = wpool.tile([P, KB, C], FP8)
        nc.sync.dma_start(qw_sb[:], w_d["qw8"].rearrange("p k c -> p k c"))
        srw_sb = wpool.tile([P, 16, KB, C], FP8)
        nc.sync.dma_start(srw_sb[:],
                          w_d["srw8"].rearrange("p t k c -> p t k c"))
        kw_sb = wpool.tile([P, KB, C], BF16)
        nc.sync.dma_start(kw_sb[:], w_d["kw"].rearrange("(k p) c -> p k c", p=P))
        vw_sb = wpool.tile([P, KB, C], BF16)
        nc.sync.dma_start(vw_sb[:], w_d["vw"].rearrange("(k p) c -> p k c", p=P))
        pjw_sb = wpool.tile([P, KB, C], FP8)
        nc.sync.dma_start(pjw_sb[:], w_d["pjw8"].rearrange("p k c -> p k c"))
        f1w_sb = wpool.tile([P, KB, HID], FP8)
        nc.sync.dma_start(f1w_sb[:], w_d["f1w8"].rearrange("p k c -> p k c"))
        f2w_sb = wpool.tile([P, MB // 2, 2, C], FP8)
        nc.sync.dma_start(f2w_sb[:],
                          w_d["f2w8"].rearrange("g p two c -> p g two c"))
        dww_sb = wpool.tile([P, MB, 9], F32)
        nc.sync.dma_start(dww_sb[:], w_d["dww"].rearrange("p m t -> p m t"))
        dwb_sb = wpool.tile([P, MB], F32)
        nc.sync.dma_start(dwb_sb[:], w_d["dwb"].rearrange("(m p) -> p m", p=P))

        bias_sb = {}
        for nm, dim in (("qb", C), ("srb", C), ("kb", C), ("f1b", HID)):
            if nz[nm]:
                t = wpool.tile([P, dim // P], F32, name=f"bias_{nm}")
                nc.sync.dma_start(t[:], w_d[nm].rearrange("(k p) -> p k", p=P))
                bias_sb[nm] = t
        for nm in ("vb", "pjb", "f2b"):
            if nz[nm]:  # free-axis bias: broadcast across partitions
                t = wpool.tile([P, C], F32, name=f"biasbc_{nm}")
                nc.sync.dma_start(t[:], w_d[nm].to_broadcast([P, C]))
                bias_sb[nm] = t

        # ========== phase A: ln1 + transpose + q =====================
        with ExitStack() as pctx:
            hcpool = pctx.enter_context(tc.tile_pool(name="hca", bufs=1))
            qa_ps = pctx.enter_context(
                tc.tile_pool(name="qaps", bufs=1, space="PSUM"))
            nc.sync.dma_start(
                xfull[:, :, 0:C],
                x_d.rearrange("(q p) c -> p q c", p=P))
            st1 = _stats_via_reduce(nc, hcpool, xfull[:, :, 0:C], eps1, "ln1")
            hc = hcpool.tile([P, TT, C], BF16, tag="hc", name="hc")
            for t in range(TT):
                nc.vector.tensor_scalar(
                    out=hc[:, t, :], in0=xfull[:, t, 0:C],
                    scalar1=st1[:, 2, t:t + 1], scalar2=st1[:, 5, t:t + 1],
                    op0=OP.subtract, op1=OP.mult)
            # one xbar transpose for all of h, then one fp8 cast
            hcW = hcpool.tile([P, TT, KB, P], BF16, tag="hcW", name="hcW")
            nc.sync.dma_start(out=hcW[:], in_=hc[:], transpose=True)
            nc.scalar.copy(
                out=hcT8[:].rearrange("p k (t f) -> p k t f", t=TT),
                in_=hcW[:].rearrange("p t k f -> p k t f"))
            # q: stationary per c-out block, 8 matmuls into 8 PSUM banks
            for cb in range(KB):
                ps = qa_ps.tile([P, 8, 512], F32, tag="qps", name="qps")
                for g in range(8):
                    nc.tensor.matmul(
                        ps[:, g, :], qw_sb[:, :, cb * P:(cb + 1) * P],
                        hcT8[:, :, g * 512:(g + 1) * 512],
                        start=True, stop=True, perf_mode=DR)
                dst = qT[:, cb, :].rearrange("p (g f) -> p g f", g=8)
                if nz["qb"]:
                    nc.vector.tensor_scalar(
                        out=dst, in0=ps[:],
                        scalar1=bias_sb["qb"][:, cb:cb + 1],
                        scalar2=None, op0=OP.add)
                else:
                    nc.scalar.copy(out=dst, in_=ps[:])

        # ========== phase B: SR-conv, srn, k, v ======================
        with ExitStack() as pctx:
            mm_ps = pctx.enter_context(
                tc.tile_pool(name="mmB", bufs=3, space="PSUM"))
            bwork = pctx.enter_context(tc.tile_pool(name="bwork", bufs=1))

            # SR conv -> hsT (feature-major [co, nk]); fp8 DR pairs the
            # two c-in chunks per tap
            hsT = bwork.tile([P, KB, NK], BF16, tag="hsT")
            conv_rhs8 = hcT8[:].rearrange(
                "p k (r a c b) -> p k a b r c", a=SR, b=SR, c=HW // SR)
            csps = mm_ps.tile([P, KB, NK], F32, tag="mmc", name="psconv")
            for cob in range(KB):
                for tap in range(16):
                    dy, dx = tap // SR, tap % SR
                    nc.tensor.matmul(
                        csps[:, cob, :],
                        srw_sb[:, tap, :, cob * P:(cob + 1) * P],
                        conv_rhs8[:, :, dy, dx, :, :],
                        start=(tap == 0), stop=(tap == 15), perf_mode=DR)
            if nz["srb"]:
                for cob in range(KB):
                    nc.vector.tensor_scalar(
                        out=hsT[:, cob, :], in0=csps[:, cob, :],
                        scalar1=bias_sb["srb"][:, cob:cob + 1],
                        scalar2=None, op0=OP.add)
            else:
                nc.vector.tensor_copy(out=hsT[:], in_=csps[:])

            # srn layernorm: batched xbar transpose -> stats -> normalize -> back
            # hs_tokW[p, cb, nkb, f] = hs[nk=nkb*128+p, c=cb*128+f]
            hs_tokW = bwork.tile([P, KB, KB, P], BF16, tag="hstok")
            nc.sync.dma_start(out=hs_tokW[:], in_=hsT[:], transpose=True)
            hsn = bwork.tile([P, KB, C], BF16, tag="hsn")
            st_s = _stats_via_reduce(
                nc, bwork, hs_tokW[:].rearrange("p c n f -> p n c f"),
                epss, "srn")
            for n in range(KB):
                nc.vector.tensor_scalar(
                    out=hsn[:, n, :].rearrange("p (c f) -> p c f", c=KB),
                    in0=hs_tokW[:, :, n, :],
                    scalar1=st_s[:, 2, n:n + 1], scalar2=st_s[:, 5, n:n + 1],
                    op0=OP.subtract, op1=OP.mult)
            # hsnW[p, nkb, cb, f] = hsn_val[nk=nkb*128+f, c=cb*128+p]
            hsnW = bwork.tile([P, KB, KB, P], BF16, tag="hsnT")
            nc.sync.dma_start(out=hsnW[:], in_=hsn[:], transpose=True)

            # k^T [c, nk]
            kps = mm_ps.tile([P, KB, NK], F32, tag="mmc", name="psk")
            for cb in range(KB):
                for kb in range(KB):
                    nc.tensor.matmul(
                        kps[:, cb, :], kw_sb[:, kb, cb * P:(cb + 1) * P],
                        hsnW[:, :, kb, :],
                        start=(kb == 0), stop=(kb == KB - 1))
            if nz["kb"]:
                for cb in range(KB):
                    nc.vector.tensor_scalar(
                        out=kT[:, cb, :], in0=kps[:, cb, :],
                        scalar1=bias_sb["kb"][:, cb:cb + 1],
                        scalar2=None, op0=OP.add)
            else:
                nc.vector.tensor_copy(out=kT[:], in_=kps[:])
            # v (token-major, (1-a) folded) -> fp8 planes; va = a/(1-a)*v
            vps = mm_ps.tile([P, KB, C], F32, tag="mmc", name="psv")
            for nkb in range(KB):
                for kb in range(KB):
                    nc.tensor.matmul(
                        vps[:, nkb, :], hsnW[:, nkb, kb, :],
                        vw_sb[:, kb, :],
                        start=(kb == 0), stop=(kb == KB - 1))
            if nz["vb"]:
                for nkb in range(KB):
                    nc.vector.tensor_add(
                        out=v8[:, nkb, :], in0=vps[:, nkb, :],
                        in1=bias_sb["vb"][:])
            else:
                nc.vector.tensor_copy(out=v8[:], in_=vps[:])

            # fp8 DR matmuls require dst partition 0: build zero-padded
            # block-diagonal stationary tiles (per-head 32-col blocks) so
            # each head's DR matmul writes the full [128,512] PSUM tile.
            # One strided write per tile: the hh dim advances by 32 cols
            # INSIDE the block as well as by one block, so its AP step is
            # block_stride + 32 (APs are linear in the indices).
            def _diag_ap(t, ghk_dims):
                base = t if isinstance(t, bass.AP) else t[:]
                ap = [list(p) for p in base.ap]
                # dims: [p, hg?, hh, kb, c32] - bump the hh step by 32
                hh_dim = len(ap) - 3
                ap[hh_dim][0] += 32
                return bass.AP(base.tensor, base.offset,
                               [tuple(p) for p in ap])

            nc.gpsimd.memset(v8bd[:], 0.0)
            nc.gpsimd.memset(onesbd[:], 0.0)
            nc.gpsimd.memset(_diag_ap(onesbd[:, :, :, 0:32], None), 1.0)
            vsrc = v8[:].rearrange("p k (g h c) -> p g h k c", g=KB, h=4)
            nc.vector.tensor_copy(
                out=_diag_ap(v8bd[:, :, :, :, 0:32], None), in_=vsrc)

        # ========== phase C: attention ===============================
        with ExitStack() as pctx:
            cwork = pctx.enter_context(tc.tile_pool(name="cwork", bufs=1))
            ot8full = cwork.tile([P, KB, N], FP8, tag="ot8full",
                                 name="ot8full")
            opsb = cwork.tile([P, KB, N], BF16, tag="opsb",
                              name="opsb")

            # op pre-phase: opsb[c, t] = (alpha * pos @ v)^T. One posD
            # buffer reused across both quads, so quad 1's matmuls repeat
            # quad 0's stationary sequence with only DMA/ACT instructions
            # between - the dedup pass keeps just 8 ldweights total.
            with ExitStack() as octx:
                opool = octx.enter_context(tc.tile_pool(name="opd",
                                                        bufs=1))
                op_ps = octx.enter_context(
                    tc.tile_pool(name="opps", bufs=1, space="PSUM"))
                posD = opool.tile([P, KB, KB, 4, 2048], FP8, tag="posD",
                                  name="posD")
                for quad in range(2):
                    qsl = slice(quad * 2048, (quad + 1) * 2048)
                    nc.sync.dma_start(posD[:], pos_d[:, :, :, :, qsl])
                    opps = op_ps.tile([P, KB, 4, 512], F32, tag="opps",
                                      name="opps")
                    hgs = range(KB) if quad == 0 else reversed(range(KB))
                    for hg in hgs:
                        hhs = range(4) if quad == 0 else reversed(range(4))
                        for hh in hhs:
                            first, last = ((0, 3) if quad == 0 else (3, 0))
                            for tq in range(4):
                                nc.tensor.matmul(
                                    opps[:, hg, tq, :], v8bd[:, hg, hh, :, :],
                                    posD[:, hg, :, hh,
                                         tq * 512:(tq + 1) * 512],
                                    start=(hh == first), stop=(hh == last),
                                    perf_mode=DR)
                    nc.scalar.copy(
                        out=opsb[:, :, qsl].rearrange(
                            "p k (q f) -> p k q f", q=4),
                        in_=opps[:])

            # --- scores (strip-resident stationaries), softmax exp, then
            # G / attn@v merged across both head-groups per quad ----------
            for quad in range(2):
                qsl = slice(quad * 2048, (quad + 1) * 2048)
                es8q = cwork.tile([P, KB, 4, KB, 2048], FP8, tag="es8q",
                                  name="es8q")
                for hg in range(KB):
                    with ExitStack() as sctx:
                        s_ps = sctx.enter_context(tc.tile_pool(
                            name="sps", bufs=1, space="PSUM"))
                        for nkb in range(KB):
                            sps = s_ps.tile([P, 4, 2, 512], F32,
                                            tag="sps", name="sps")
                            for half in range(2):
                                for hh in range(4):
                                    hb = slice(32 * hh, 32 * (hh + 1))
                                    for tg in range(2):
                                        t0 = (quad * 4 + half * 2
                                              + tg) * 512
                                        nc.tensor.matmul(
                                            sps[:, hh, tg, :],
                                            kT[hb, hg,
                                               nkb * P:(nkb + 1) * P],
                                            qT[hb, hg, t0:t0 + 512],
                                            start=True, stop=True,
                                            tile_position=(32 * hh, 0))
                                hsl = slice(half * 1024, (half + 1) * 1024)
                                nc.scalar.activation(
                                    es8q[:, hg, :, nkb, hsl]
                                    .rearrange("p h (g f) -> p h g f",
                                               g=2),
                                    sps[:], AF.Exp, scale=scale)
                with ExitStack() as goctx:
                  gwork = goctx.enter_context(
                      tc.tile_pool(name="gwork", bufs=1))
                  with ExitStack() as gctx:
                    g_ps = gctx.enter_context(tc.tile_pool(
                        name="gps", bufs=1, space="PSUM"))
                    gps = g_ps.tile([P, KB, 4, 512], F32, tag="gps",
                                    name="gps")
                    for hh in range(4):
                        for hg in range(KB):
                            for tq in range(4):
                                nc.tensor.matmul(
                                    gps[:, hg, tq, :], onesbd[:, hh, :, :],
                                    es8q[:, hg, hh, :,
                                         tq * 512:(tq + 1) * 512],
                                    start=(hh == 0), stop=(hh == 3),
                                    perf_mode=DR)
                    gsb = gwork.tile([P, KB, 4, 512], F32, tag="gsb",
                                     name="gsb")
                    nc.vector.reciprocal(gsb[:], gps[:])
                  with ExitStack() as gctx:
                    oe_ps = gctx.enter_context(tc.tile_pool(
                        name="oeps", bufs=1, space="PSUM"))
                    oeps = oe_ps.tile([P, KB, 4, 512], F32, tag="oeps",
                                      name="oeps")
                    for hh in range(4):
                        for hg in range(KB):
                            for tq in range(4):
                                nc.tensor.matmul(
                                    oeps[:, hg, tq, :], v8bd[:, hg, hh, :, :],
                                    es8q[:, hg, hh, :,
                                         tq * 512:(tq + 1) * 512],
                                    start=(hh == 0), stop=(hh == 3),
                                    perf_mode=DR)
                    tmpc = gwork.tile([P, KB, 4, 512], BF16, tag="tmpc",
                                      name="tmpc")
                    nc.vector.tensor_tensor(
                        out=tmpc[:], in0=oeps[:], in1=gsb[:], op=OP.mult)
                    nc.vector.scalar_tensor_tensor(
                        out=ot8full[:, :, qsl].rearrange(
                            "p k (q f) -> p k q f", q=4),
                        in0=opsb[:, :, qsl].rearrange(
                            "p k (q f) -> p k q f", q=4),
                        scalar=av_s, in1=tmpc[:],
                        op0=OP.mult, op1=OP.add)

            # --- proj, computed transposed (pT[c,t]), then one batched
            # xbar transpose per c-block and one residual add -------------
            with ExitStack() as pjctx:
                pjpool = pjctx.enter_context(tc.tile_pool(name="pjp",
                                                          bufs=1))
                pj_ps = pjctx.enter_context(tc.tile_pool(
                    name="pjps", bufs=1, space="PSUM"))
                for cb in range(KB):
                    pjps = pj_ps.tile([P, 8, 512], F32, tag="pjps",
                                      name="pjps")
                    for t8 in range(8):
                        nc.tensor.matmul(
                            pjps[:, t8, :], pjw_sb[:, :, cb * P:(cb + 1) * P],
                            ot8full[:, :, t8 * 512:(t8 + 1) * 512],
                            start=True, stop=True, perf_mode=DR)
                    pjsb = pjpool.tile([P, 8, 512], BF16, tag="pjsb",
                                       name="pjsb")
                    nc.scalar.copy(out=pjsb[:], in_=pjps[:])
                    pjT = pjpool.tile([P, TT, P], BF16, tag="pjT",
                                      name="pjT")
                    nc.sync.dma_start(out=pjT[:], in_=pjsb[:],
                                      transpose=True)
                    nc.vector.tensor_tensor(
                        out=x2[:, :, cb * P:(cb + 1) * P],
                        in0=xfull[:, :, cb * P:(cb + 1) * P],
                        in1=pjT[:], op=OP.add)
            if nz["pjb"]:
                for tt in range(TT):
                    nc.vector.tensor_add(
                        out=x2[:, tt, 0:C], in0=x2[:, tt, 0:C],
                        in1=bias_sb["pjb"][:])

        # ---- ln2 + h2^T (batched stats, one transpose, one fp8 cast) ----
        with ExitStack() as l2ctx:
            l2pool = l2ctx.enter_context(tc.tile_pool(name="l2p", bufs=1))
            st2 = _stats_via_reduce(nc, l2pool, x2[:, :, 0:C], eps1, "ln2")
            h2c = l2pool.tile([P, TT, C], BF16, tag="h2c", name="h2c")
            for t in range(TT):
                nc.vector.tensor_scalar(
                    out=h2c[:, t, :], in0=x2[:, t, 0:C],
                    scalar1=st2[:, 2, t:t + 1], scalar2=st2[:, 5, t:t + 1],
                    op0=OP.subtract, op1=OP.mult)
            h2W = l2pool.tile([P, TT, KB, P], BF16, tag="h2W", name="h2W")
            nc.sync.dma_start(out=h2W[:], in_=h2c[:], transpose=True)
            nc.scalar.copy(
                out=h2T8[:].rearrange("p k (t f) -> p k t f", t=TT),
                in_=h2W[:].rearrange("p t k f -> p k t f"))

        # ========== phase D: MLP =====================================
        with ExitStack() as pctx:
            m2cp = pctx.enter_context(tc.tile_pool(name="m2c", bufs=1))
            accp = pctx.enter_context(tc.tile_pool(name="accd", bufs=2))

            # two rotating padded layouts; borders zeroed once
            mpads = [m2cp.tile([P, NPAD], BF16, tag=f"mpad{j}",
                               name=f"mpad{j}")
                     for j in range(2)]
            for mp in mpads:
                nc.gpsimd.memset(mp[:], 0.0)

            m2pairs = []
            with ExitStack() as f1ctx:
                mm_ps = f1ctx.enter_context(
                    tc.tile_pool(name="mmD", bufs=1, space="PSUM"))
                for mb in range(MB):
                    mpad = mpads[mb % 2]
                    vp = mpad[:].rearrange("p (r c) -> p r c", c=PADW)
                    # one stationary, 8 matmuls into all 8 PSUM banks
                    ps = mm_ps.tile([P, 8, 512], F32, tag="mmd", name="psf1")
                    for nt in range(8):
                        nc.tensor.matmul(
                            ps[:, nt, :],
                            f1w_sb[:, :, mb * P:(mb + 1) * P],
                            h2T8[:, :, nt * 512:(nt + 1) * 512],
                            start=True, stop=True, perf_mode=DR)
                    dst = vp[:, 1:65, 1:65].rearrange(
                        "p (a r) c -> p a r c", a=8)
                    src = ps.rearrange("p k (r c) -> p k r c", c=HW)
                    if nz["f1b"]:
                        nc.scalar.tensor_scalar(
                            out=dst, in0=src,
                            scalar1=bias_sb["f1b"][:, mb:mb + 1],
                            scalar2=None, op0=OP.add)
                    else:
                        nc.scalar.copy(out=dst, in_=src)
                    # depthwise 3x3 conv: 9 accumulations (DVE + Pool split)
                    acc = accp.tile([P, HW * HW], BF16, tag="acc",
                                    name=f"acc{mb}")
                    av = acc[:].rearrange("p (r c) -> p r c", c=HW)
                    nc.vector.tensor_scalar(
                        out=av[:], in0=vp[:, 0:HW, 0:HW],
                        scalar1=dww_sb[:, mb, 0:1], scalar2=None, op0=OP.mult)
                    for tap in range(1, 9):
                        dy, dx = tap // 3, tap % 3
                        nc.vector.scalar_tensor_tensor(
                            out=av[:], in0=vp[:, dy:dy + HW, dx:dx + HW],
                            scalar=dww_sb[:, mb, tap:tap + 1], in1=av[:],
                            op0=OP.mult, op1=OP.add)
                    if mb % 2 == 0:
                        m2pair = m2cp.tile([P, 2, N], FP8, tag=f"m2c{mb // 2}",
                                           name=f"m2pair{mb}")
                        m2pairs.append(m2pair)
                    nc.scalar.activation(
                        m2pair[:, mb % 2, :].rearrange("p (r c) -> p r c",
                                                       c=HW),
                        av[:], AF.Gelu, bias=dwb_sb[:, mb:mb + 1])
            # fc2 transposed (yT[c,t], fp8 DR over all 4 hid pair-groups),
            # stationary-outer: per c-block, 4 ldweights feed 32 matmuls
            # across all 8 token groups (8 PSUM banks), then one eviction,
            # one batched transpose and one residual add.
            with ExitStack() as f2ctx:
                ypool = f2ctx.enter_context(tc.tile_pool(name="yp", bufs=1))
                f2_ps = f2ctx.enter_context(
                    tc.tile_pool(name="f2ps", bufs=1, space="PSUM"))
                for cb in range(KB):
                    yps = f2_ps.tile([P, 8, 512], F32, tag="fps", name="yps")
                    for g in range(4):
                        for ttg in range(8):
                            nc.tensor.matmul(
                                yps[:, ttg, :],
                                f2w_sb[:, g, :, cb * P:(cb + 1) * P],
                                m2pairs[g][:, :, ttg * 512:(ttg + 1) * 512],
                                start=(g == 0), stop=(g == 3), perf_mode=DR)
                    yt = ypool.tile([P, 8, 512], BF16, tag="yt", name="yt")
                    nc.scalar.copy(out=yt[:], in_=yps[:])
                    y4 = ypool.tile([P, TT, P], BF16, tag="y4", name="y4")
                    nc.sync.dma_start(out=y4[:], in_=yt[:], transpose=True)
                    nc.vector.tensor_tensor(
                        out=x2[:, :, cb * P:(cb + 1) * P],
                        in0=x2[:, :, cb * P:(cb + 1) * P],
                        in1=y4[:], op=OP.add)
            if nz["f2b"]:
                for tt in range(TT):
                    nc.vector.tensor_add(
                        out=x2[:, tt, 0:C], in0=x2[:, tt, 0:C],
                        in1=bias_sb["f2b"][:])

            nc.sync.dma_start(
                out_d.rearrange("(q p) c -> p q c", p=P),
                x2[:, :, 0:C])

    return nc


def _prep_pos(pos_b):
    """pos[H, N, NK] f32 -> [p, hg, nkb, hh, N] fp8 (transposed per head)."""
    arr = pos_b.reshape(KB, 4, N, KB, P).transpose(4, 0, 3, 1, 2)
    return _f8(arr)


def _run(inputs, trace=False):
    a = float(np.asarray(inputs["alpha"]).reshape(-1)[0])
    w = _prep_weights(inputs, a)
    nz = {nm: bool(np.any(w[nm])) for nm in
          ("qb", "srb", "kb", "vb", "pjb", "f1b", "f2b")}
    nc = _build_program(a, nz)
    _move_matmul_waits(nc)
    _dedup_ldweights(nc)
    _merge_waits(nc)
    _split_drain_waits(nc)

    x = np.asarray(inputs["x"], np.float32)
    pos = np.asarray(inputs["pos_2D"], np.float32)
    shared = {k: v for k, v in w.items()
              if k in ("qw8", "srw8", "kw", "vw", "pjw8", "f1w8", "dww",
                       "dwb", "f2w8")}
    for nm in ("qb", "srb", "kb", "vb", "pjb", "f1b", "f2b"):
        if nz[nm]:
            shared[nm] = w[nm]
    in_maps = []
    for b in range(B):
        in_maps.append(dict(shared, x=np.ascontiguousarray(x[b]),
                            pos8=_prep_pos(pos[b])))
    res = run_bass_kernel_spmd(nc, in_maps, list(range(B)), trace=trace)
    out = np.stack([res.results[b]["out"] for b in range(B)]).astype(np.float32)
    return out, res


def kernel(**inputs) -> np.ndarray:
    out, _ = _run(inputs, trace=False)
    return out



# revision 50
# speedup vs baseline: 1.0131x; 1.0131x over previous
"""Trainium2 Bass kernel for nn_Block_13391708030014 (dense transformer block).

Sharding: data-parallel over batch — core b computes batch item b entirely
(B == n_cores == 8), no collectives.

The target runtime dispatches instructions with a large fixed per-instruction
cost (engines effectively serialized), so the design minimizes TOTAL
instruction count (~1.1k bass / ~1.4k NEFF vs 2.0k/2.4k for the previous
iteration, which itself was down from ~7.6k). Matmul count (520) sits at the
PSUM-output/contraction floor for this decomposition; everything else is
batched into as few giant instructions as the ISA allows:

  A. x loaded in ONE DMA; ln1 stats for ALL 32 token tiles in ~9 ops
     (Square on ACT + two inner-axis tensor_reduce + small fixups);
     32 per-tile normalizes (per-partition scalar limit); ONE batched
     xbar DMA-transpose for all of h ([128, 16KB] -> 64 blocks) + ONE
     fp8 cast; q as 16 DR matmuls under 2 ldweights (8 PSUM banks each,
     single whole-PSUM evictions).
  B. SR conv: 16 taps x 2 c-chunks as 32 DR matmuls on strided views of
     h^T(fp8); srn stats via reduce; one batched transpose each way; the
     block-diagonal v8bd/onesbd stationaries built with ONE strided
     copy each (hh AP step = block_stride+32); (1-alpha) folded into vw.
  C. pos@v pre-phase: pos loaded in ONE DMA per 2048-token quad into a
     reused buffer, (1-a)*pos@v accumulated via v8bd (4 heads into 8
     banks, quad-1 stationary order reversed for one extra dedup), ONE
     eviction per quad; the a/(1-a) ratio is applied later inside the
     combine scalar_tensor_tensor, sparing a second fp8 quantization of
     v. per quad: scores TRANSPOSED (sT[nk,t] = k_h^T q_h): the
     PE's four 32-row strips hold the 4 heads' k stationaries
     INDEPENDENTLY (tile_position row groups - a 32-row ldweights only
     clobbers its own strip), so per nk-block the 4 stationaries load
     once and feed all 4 token chunks (8 ldweights + 4 Exps per
     (quad, head-group)); G and attn@v as DR matmuls with block-diagonal
     stationaries merged across BOTH head-groups per quad (shared onesbd
     ldweights, 8-bank accumulators, ONE reciprocal + two tensor ops per
     quad). proj computed TRANSPOSED (pT[c,t], 8 matmuls per c-block
     under 1 ldweights), one eviction + one batched transpose + one
     residual add per c-block; ln2 like phase A.
  D. fc1: per hidden block ONE ldweights + 8 DR matmuls into all 8 PSUM
     banks, ONE eviction into the zero-padded 66x66 spatial layout;
     depthwise 3x3 conv as 9 scalar_tensor_tensor chains on DVE;
     bias+Gelu fused into one fp8 eviction per block; fc2 TRANSPOSED
     with stationary-outer loops (4 ldweights per c-block feed 32
     matmuls into 8 banks), ONE eviction/transpose/residual-add per
     c-block; output stored in ONE DMA.

Cross-cutting passes (in _run): matmul waits folded onto ldweights
(bass_rust); consecutive same-stationary InstLdweights removed (PE array
keeps weights across matmuls - verified on HW), their waits reattached to
the following matmul; same-semaphore waits merged to the max value;
remaining multi-wait instructions split onto 2-wait EventSemaphore NOPs
(walrus 1-wait limit). HWDGE DMA completion collapsed to one sem lane.
Stats/x tiles stride-padded so the AP optimizer cannot merge token groups.
Hardware rel err ~7.4e-3 (fp8 noise; tolerance 2e-2).
"""

from contextlib import ExitStack

import numpy as np
import ml_dtypes

import concourse.bass as bass
import concourse.tile as tile
from concourse import mybir
from concourse.bass_utils import run_bass_kernel_spmd

F32 = mybir.dt.float32
BF16 = mybir.dt.bfloat16
FP8 = mybir.dt.float8e4
AF = mybir.ActivationFunctionType
OP = mybir.AluOpType
DR = mybir.MatmulPerfMode.DoubleRow

B, N, C = 8, 4096, 256
H, DH = 8, 32
NK = 256
HID = 1024
HW = 64
SR = 4
P = 128
TT = N // P          # 32 token tiles
KB = C // P          # 2 channel blocks
MB = HID // P        # 8 hidden blocks
PADW = HW + 2        # 66
CP = C + 1           # stride-padded token row (prevents AP dim-merge)
NPAD = PADW * PADW   # 4356


def _split_drain_waits(nc, max_waits=1, dma_only=False):
    """walrus refuses >1 sem wait per instruction (2 on InstEventSemaphore).
    Keep the first wait on the instruction and hoist the rest, packed in
    pairs, onto InstEventSemaphore instructions inserted just before it on
    the same engine (semantically identical: same engine, program order).
    dma_only=True limits splitting to DMA-ish instructions (experiment:
    walrus appears to lower compute-instruction waits as standalone
    SEMAPHORE ops anyway)."""
    dma_types = ("InstDMACopy", "InstDmaTransposeAnt", "InstDrain",
                 "InstTensorLoad", "InstTensorSave")
    for f in nc.m.functions:
        for blk in f.blocks:
            insts = blk.instructions
            new = []
            changed = False
            for inst in insts:
                si = getattr(inst, "sync_info", None)
                if dma_only and type(inst).__name__ not in dma_types:
                    new.append(inst)
                    continue
                if si is not None and si.on_wait and len(si.on_wait) > max_waits:
                    waits = list(si.on_wait)
                    extra = waits[max_waits:]
                    for i in range(0, len(extra), 2):
                        new.append(mybir.InstEventSemaphore(
                            name=f"{inst.name}-ws{i}",
                            sync_info=mybir.SyncInfo(
                                on_wait=extra[i:i + 2], on_update=[]),
                            bass_nofuse=True,
                            engine=inst.engine,
                            ins=[], outs=[],
                        ))
                    inst.sync_info = mybir.SyncInfo(
                        on_wait=waits[:max_waits],
                        on_update=list(si.on_update or []))
                    changed = True
                new.append(inst)
            if changed:
                blk.instructions = new


def _move_matmul_waits(nc):
    """Fold matmul waits onto the paired ldweights (no extra instructions)."""
    try:
        import bass_rust
        bass_rust.move_matmul_waits_to_ldweights(nc.m)
    except Exception:
        pass


def _merge_waits(nc):
    """Merge sem-ge-imm waits on the same semaphore: keep the max value.
    (Waits are monotone >= conditions, so the max implies the rest.)"""
    for f in nc.m.functions:
        for blk in f.blocks:
            for inst in blk.instructions:
                si = getattr(inst, "sync_info", None)
                if si is None or not si.on_wait or len(si.on_wait) < 2:
                    continue
                best, order, rest = {}, [], []
                for w in si.on_wait:
                    if (w.sync_type == "semaphore"
                            and w.wait_mode == "sem-ge-imm"
                            and w.wait_reg is None):
                        if w.id not in best:
                            best[w.id] = w
                            order.append(w.id)
                        elif w.wait_value > best[w.id].wait_value:
                            best[w.id] = w
                    else:
                        rest.append(w)
                merged = [best[k] for k in order] + rest
                if len(merged) < len(si.on_wait):
                    inst.sync_info = mybir.SyncInfo(
                        on_wait=merged, on_update=list(si.on_update or []))


# SBUF tiles that are written once (before any dependent ldweights) and then
# only read: safe targets for ldweights dedup.
_LDW_STABLE_PREFIXES = (
    "qw_sb", "srw_sb", "kw_sb", "vw_sb", "pjw_sb", "f1w_sb", "f2w_sb",
    "kT", "v8bd", "va8bd", "onesbd",
)


def _dedup_ldweights(nc):
    """Remove an InstLdweights when the immediately preceding PE ldweights
    loaded the identical stationary (same AP/perf_mode/tile_position) and the
    tile is write-once (whitelist). The PE array keeps weights across matmuls,
    so the duplicate load is redundant. Any waits on the removed instruction
    move to the next PE instruction (its matmul) - program order on the PE
    engine is unchanged, so semantics are preserved."""
    PE = mybir.EngineType.PE
    n_removed = 0
    for f in nc.m.functions:
        for blk in f.blocks:
            insts = blk.instructions
            # indices of PE instructions in stream order
            pe_idx = [i for i, inst in enumerate(insts)
                      if getattr(inst, "engine", None) == PE]
            drop = set()
            pending_waits = {}  # target stream index -> list of waits
            # The 128x128 PE array is 4 independent 32-row strips
            # (tile_position row groups); a 32-row ldweights only
            # clobbers its own strip, so track the resident stationary
            # per strip.
            strip_key = [None] * 4
            for j, i in enumerate(pe_idx):
                inst = insts[i]
                nm = type(inst).__name__
                if nm == "InstLdweights":
                    key = (repr(inst.ins), repr(inst.perf_mode),
                           repr(inst.tile_position),
                           repr(getattr(inst, "is_transpose", None)))
                    tp = getattr(inst, "tile_position", None)
                    ts_ = getattr(inst, "tile_size", None)
                    r0 = tp[0] if tp else 0
                    nr = ts_[0] if ts_ else 128
                    strips = range(r0 // 32, min(4, (r0 + nr + 31) // 32))
                    stable = any(p in repr(inst.ins)
                                 for p in _LDW_STABLE_PREFIXES)
                    if (stable and j + 1 < len(pe_idx)
                            and all(strip_key[s] == key for s in strips)):
                        si = getattr(inst, "sync_info", None)
                        if si is not None and si.on_wait:
                            tgt = pe_idx[j + 1]
                            pending_waits.setdefault(tgt, []).extend(
                                si.on_wait)
                        drop.add(i)
                        n_removed += 1
                        continue
                    for s in strips:
                        strip_key[s] = key
                elif nm == "InstMatmult":
                    pass  # does not clobber loaded weights
                elif nm in ("InstEventSemaphore", "InstDrain", "InstNop"):
                    pass  # no effect on the PE array
                else:
                    strip_key = [None] * 4  # unknown PE instr: be safe
            if not drop:
                continue
            for tgt, waits in pending_waits.items():
                inst = insts[tgt]
                si = getattr(inst, "sync_info", None)
                old = list(si.on_wait) if si is not None and si.on_wait else []
                upd = list(si.on_update or []) if si is not None else []
                inst.sync_info = mybir.SyncInfo(on_wait=old + waits,
                                                on_update=upd)
            blk.instructions = [inst for i, inst in enumerate(insts)
                                if i not in drop]
    return n_removed


def _bf(x):
    return np.ascontiguousarray(x.astype(ml_dtypes.bfloat16))


def _f8(x):
    return np.ascontiguousarray(x.astype(ml_dtypes.float8_e4m3))


def _prep_weights(i, a):
    """Fold LN affines + (1-alpha) into weights; return DRAM payloads."""
    ln1_w, ln1_b = i["ln1_w"], i["ln1_b"]
    ln2_w, ln2_b = i["ln2_w"], i["ln2_b"]

    qw = ln1_w[:, None] * i["q_w"]                      # [C, C]
    qb = ln1_b @ i["q_w"] + i["q_b"]                    # [C]

    # sr_w is OIHW: [c_out, c_in, dy, dx] -> srw[tap, ci, co]
    srw = (i["sr_w"] * ln1_w[None, :, None, None]).transpose(2, 3, 1, 0)
    srw = np.ascontiguousarray(srw.reshape(SR * SR, C, C))
    srb = i["sr_b"] + np.einsum("i,oihw->o", ln1_b, i["sr_w"])

    srn_w, srn_b = i["srn_w"], i["srn_b"]
    kvw = srn_w[:, None] * i["kv_w"]                    # [C, 2C]
    kvb = srn_b @ i["kv_w"] + i["kv_b"]
    kw, vw = kvw[:, :C], kvw[:, C:]
    kb_, vb = kvb[:C], kvb[C:]
    # fold (1-a) into the v weights (the softmax path); the pos path then
    # multiplies by a/(1-a) to recover alpha*v.
    vw1 = (1.0 - a) * vw
    vb1 = (1.0 - a) * vb

    f1w = ln2_w[:, None] * i["fc1_w"]                   # [C, HID]
    f1b = ln2_b @ i["fc1_w"] + i["fc1_b"]

    dww = i["dw_w"].reshape(HID, 9)                     # [HID, tap]
    # [128, MB, 9] per-partition scalars
    dww_p = np.ascontiguousarray(
        dww.reshape(MB, P, 9).transpose(1, 0, 2))

    # fc2 as fp8 DoubleRow over hidden-block pairs: [4, 128, 2, C]
    f2w8 = np.ascontiguousarray(
        i["fc2_w"].reshape(MB // 2, 2, P, C).transpose(0, 2, 1, 3))

    # proj fp8 DoubleRow over c-chunk pairs: [128, 2, C]
    pjw8 = np.ascontiguousarray(
        i["proj_w"].reshape(KB, P, C).transpose(1, 0, 2))

    # fp8 DoubleRow layouts pairing the two c-in chunks: [128, 2, out]
    qw8 = np.ascontiguousarray(qw.reshape(KB, P, C).transpose(1, 0, 2))
    srw8 = np.ascontiguousarray(
        srw.reshape(16, KB, P, C).transpose(2, 0, 1, 3))  # [128, 16, 2, C]
    f1w8 = np.ascontiguousarray(f1w.reshape(KB, P, HID).transpose(1, 0, 2))

    return {
        "qw8": _f8(qw8), "qb": qb.astype(np.float32),
        "srw8": _f8(srw8), "srb": srb.astype(np.float32),
        "kw": _bf(kw), "kb": kb_.astype(np.float32),
        "vw": _bf(vw1), "vb": vb1.astype(np.float32),
        "pjw8": _f8(pjw8), "pjb": i["proj_b"].astype(np.float32),
        "f1w8": _f8(f1w8), "f1b": f1b.astype(np.float32),
        "dww": dww_p.astype(np.float32),
        "dwb": i["dw_b"].astype(np.float32),
        "f2w8": _f8(f2w8), "f2b": i["fc2_b"].astype(np.float32),
    }


def _build_program(a, nz):
    # Collapse HWDGE DMA completion tracking to one sem lane: all DMAs issue
    # from SP (one FIFO ring), so cumulative single-sem waits are safe, and
    # consumers of multi-DMA regions then need 1 wait instead of up to 8
    # (the target runtime charges a fixed ~5us per instruction, and every
    # extra wait becomes an extra instruction).
    import concourse.tile_sem_assignment as _tsa
    _saved_sems = _tsa.NUM_HWDGE_SEMS
    _tsa.NUM_HWDGE_SEMS = 1
    try:
        return _build_program_inner(a, nz)
    finally:
        _tsa.NUM_HWDGE_SEMS = _saved_sems


def _stats_via_reduce(nc, pool, src_ap, eps_tile, tag):
    """Batched LN stats: per-group mean + inv-std over the innermost free
    dim(s) of src_ap [128, G, inner...] in ~9 instructions regardless of G.
    Returns the stats tile; mean at [:, 2, g], inv-std at [:, 5, g]."""
    shp = src_ap.shape
    G = shp[1]
    inner = list(shp[2:])
    nelem = 1
    for d in inner:
        nelem *= d
    axis = mybir.AxisListType.X if len(inner) == 1 else mybir.AxisListType.XY
    sq = pool.tile([P, G, nelem + 8], BF16, tag=f"sq_{tag}",
                   name=f"sq_{tag}", bufs=1)
    sqv = sq[:, :, 0:nelem]
    if len(inner) == 2:
        sqv = sqv.rearrange("p g (a b) -> p g a b", a=inner[0])
    nc.scalar.activation(sqv, src_ap, AF.Square)
    st = pool.tile([P, 6, G + 1], F32, tag=f"st_{tag}", name=f"st_{tag}",
                   bufs=1)
    nc.vector.tensor_reduce(out=st[:, 0, 0:G], in_=src_ap, axis=axis,
                            op=OP.add)
    nc.vector.tensor_reduce(out=st[:, 1, 0:G], in_=sqv, axis=axis, op=OP.add)
    nc.vector.tensor_scalar(out=st[:, 2, 0:G], in0=st[:, 0, 0:G],
                            scalar1=1.0 / nelem, scalar2=None, op0=OP.mult)
    nc.vector.tensor_tensor(out=st[:, 3, 0:G], in0=st[:, 2, 0:G],
                            in1=st[:, 2, 0:G], op=OP.mult)
    # var = s2/nelem - mu^2 in one scalar_tensor_tensor
    nc.vector.scalar_tensor_tensor(
        out=st[:, 4, 0:G], in0=st[:, 1, 0:G], scalar=1.0 / nelem,
        in1=st[:, 3, 0:G], op0=OP.mult, op1=OP.subtract)
    nc.scalar.activation(st[:, 5, 0:G], st[:, 4, 0:G], AF.Sqrt,
                         bias=eps_tile[:])
    nc.vector.reciprocal(st[:, 5, 0:G], st[:, 5, 0:G])
    return st


def _build_program_inner(a, nz):
    nc = bass.Bass("TRN2", target_bir_lowering=False, debug=False,
                   num_devices=B)

    x_d = nc.dram_tensor("x", [N, C], F32, kind="ExternalInput").ap()
    # pos, host-packed to [p(nk%128), hg, nkb, hh, N] fp8
    pos_d = nc.dram_tensor("pos8", [P, KB, KB, 4, N], FP8,
                           kind="ExternalInput").ap()
    out_d = nc.dram_tensor("out", [N, C], F32, kind="ExternalOutput").ap()

    w_d = {}
    wshapes = {
        "qw8": ([P, KB, C], FP8), "srw8": ([P, 16, KB, C], FP8),
        "kw": ([C, C], BF16), "vw": ([C, C], BF16),
        "pjw8": ([P, KB, C], FP8), "f1w8": ([P, KB, HID], FP8),
        "dww": ([P, MB, 9], F32), "dwb": ([HID], F32),
        "f2w8": ([MB // 2, P, 2, C], FP8),
    }
    for nm in ("qb", "srb", "kb", "vb", "pjb", "f1b", "f2b"):
        if nz[nm]:
            wshapes[nm] = ([{"f1b": HID}.get(nm, C)], F32)
    for nm, (shp, dt) in wshapes.items():
        w_d[nm] = nc.dram_tensor(nm, shp, dt, kind="ExternalInput").ap()

    scale = DH ** -0.5
    av_s = a / (1.0 - a) if abs(1.0 - a) > 1e-12 else 0.0

    with tile.TileContext(nc) as tc, ExitStack() as ctx:
        persist = ctx.enter_context(tc.tile_pool(name="persist", bufs=1))
        wpool = ctx.enter_context(tc.tile_pool(name="weights", bufs=1))
        stat = ctx.enter_context(tc.tile_pool(name="stat", bufs=4))

        # ---- persistent tiles
        hcT8 = persist.tile([P, KB, N], FP8, tag="hcT8")
        qT = persist.tile([P, KB, N], BF16, tag="qT")
        kT = persist.tile([P, KB, NK], BF16, tag="kT")
        v8 = persist.tile([P, KB, C], FP8, tag="v8")
        v8bd = persist.tile([P, KB, 4, KB, P], FP8, tag="v8bd")
        onesbd = persist.tile([P, 4, KB, P], FP8, tag="onesbd")
        xfull = persist.tile([P, TT, CP], F32, tag="xfull")
        x2 = persist.tile([P, TT, CP], F32, tag="x2")
        h2T8 = persist.tile([P, KB, N], FP8, tag="h2T8")

        eps1 = persist.tile([P, 1], F32, tag="eps1")
        nc.vector.memset(eps1[:], 1e-6)
        epss = persist.tile([P, 1], F32, tag="epss")
        nc.vector.memset(epss[:], 1e-5)

        # ---- weights to SBUF
        qw_sb = wpool.tile([P, KB, C], FP8)
        nc.sync.dma_start(qw_sb[:], w_d["qw8"].rearrange("p k c -> p k c"))
        srw_sb = wpool.tile([P, 16, KB, C], FP8)
        nc.sync.dma_start(srw_sb[:],
                          w_d["srw8"].rearrange("p t k c -> p t k c"))
        kw_sb = wpool.tile([P, KB, C], BF16)
        nc.sync.dma_start(kw_sb[:], w_d["kw"].rearrange("(k p) c -> p k c", p=P))
        vw_sb = wpool.tile([P, KB, C], BF16)
        nc.sync.dma_start(vw_sb[:], w_d["vw"].rearrange("(k p) c -> p k c", p=P))
        pjw_sb = wpool.tile([P, KB, C], FP8)
        nc.sync.dma_start(pjw_sb[:], w_d["pjw8"].rearrange("p k c -> p k c"))
        f1w_sb = wpool.tile([P, KB, HID], FP8)
        nc.sync.dma_start(f1w_sb[:], w_d["f1w8"].rearrange("p k c -> p k c"))
        f2w_sb = wpool.tile([P, MB // 2, 2, C], FP8)
        nc.sync.dma_start(f2w_sb[:],
                          w_d["f2w8"].rearrange("g p two c -> p g two c"))
        dww_sb = wpool.tile([P, MB, 9], F32)
        nc.sync.dma_start(dww_sb[:], w_d["dww"].rearrange("p m t -> p m t"))
        dwb_sb = wpool.tile([P, MB], F32)
        nc.sync.dma_start(dwb_sb[:], w_d["dwb"].rearrange("(m p) -> p m", p=P))

        bias_sb = {}
        for nm, dim in (("qb", C), ("srb", C), ("kb", C), ("f1b", HID)):
            if nz[nm]:
                t = wpool.tile([P, dim // P], F32, name=f"bias_{nm}")
                nc.sync.dma_start(t[:], w_d[nm].rearrange("(k p) -> p k", p=P))
                bias_sb[nm] = t
        for nm in ("vb", "pjb", "f2b"):
            if nz[nm]:  # free-axis bias: broadcast across partitions
                t = wpool.tile([P, C], F32, name=f"biasbc_{nm}")
                nc.sync.dma_start(t[:], w_d[nm].to_broadcast([P, C]))
                bias_sb[nm] = t

        # ========== phase A: ln1 + transpose + q =====================
        with ExitStack() as pctx:
            hcpool = pctx.enter_context(tc.tile_pool(name="hca", bufs=1))
            qa_ps = pctx.enter_context(
                tc.tile_pool(name="qaps", bufs=1, space="PSUM"))
            nc.sync.dma_start(
                xfull[:, :, 0:C],
                x_d.rearrange("(q p) c -> p q c", p=P))
            st1 = _stats_via_reduce(nc, hcpool, xfull[:, :, 0:C], eps1, "ln1")
            hc = hcpool.tile([P, TT, C], BF16, tag="hc", name="hc")
            for t in range(TT):
                nc.vector.tensor_scalar(
                    out=hc[:, t, :], in0=xfull[:, t, 0:C],
                    scalar1=st1[:, 2, t:t + 1], scalar2=st1[:, 5, t:t + 1],
                    op0=OP.subtract, op1=OP.mult)
            # one xbar transpose for all of h, then one fp8 cast
            hcW = hcpool.tile([P, TT, KB, P], BF16, tag="hcW", name="hcW")
            nc.sync.dma_start(out=hcW[:], in_=hc[:], transpose=True)
            nc.scalar.copy(
                out=hcT8[:].rearrange("p k (t f) -> p k t f", t=TT),
                in_=hcW[:].rearrange("p t k f -> p k t f"))
            # q: stationary per c-out block, 8 matmuls into 8 PSUM banks
            for cb in range(KB):
                ps = qa_ps.tile([P, 8, 512], F32, tag="qps", name="qps")
                for g in range(8):
                    nc.tensor.matmul(
                        ps[:, g, :], qw_sb[:, :, cb * P:(cb + 1) * P],
                        hcT8[:, :, g * 512:(g + 1) * 512],
                        start=True, stop=True, perf_mode=DR)
                dst = qT[:, cb, :].rearrange("p (g f) -> p g f", g=8)
                if nz["qb"]:
                    nc.vector.tensor_scalar(
                        out=dst, in0=ps[:],
                        scalar1=bias_sb["qb"][:, cb:cb + 1],
                        scalar2=None, op0=OP.add)
                else:
                    nc.scalar.copy(out=dst, in_=ps[:])

        # ========== phase B: SR-conv, srn, k, v ======================
        with ExitStack() as pctx:
            mm_ps = pctx.enter_context(
                tc.tile_pool(name="mmB", bufs=3, space="PSUM"))
            bwork = pctx.enter_context(tc.tile_pool(name="bwork", bufs=1))

            # SR conv -> hsT (feature-major [co, nk]); fp8 DR pairs the
            # two c-in chunks per tap
            hsT = bwork.tile([P, KB, NK], BF16, tag="hsT")
            conv_rhs8 = hcT8[:].rearrange(
                "p k (r a c b) -> p k a b r c", a=SR, b=SR, c=HW // SR)
            csps = mm_ps.tile([P, KB, NK], F32, tag="mmc", name="psconv")
            for cob in range(KB):
                for tap in range(16):
                    dy, dx = tap // SR, tap % SR
                    nc.tensor.matmul(
                        csps[:, cob, :],
                        srw_sb[:, tap, :, cob * P:(cob + 1) * P],
                        conv_rhs8[:, :, dy, dx, :, :],
                        start=(tap == 0), stop=(tap == 15), perf_mode=DR)
            if nz["srb"]:
                for cob in range(KB):
                    nc.vector.tensor_scalar(
                        out=hsT[:, cob, :], in0=csps[:, cob, :],
                        scalar1=bias_sb["srb"][:, cob:cob + 1],
                        scalar2=None, op0=OP.add)
            else:
                nc.vector.tensor_copy(out=hsT[:], in_=csps[:])

            # srn layernorm: batched xbar transpose -> stats -> normalize -> back
            # hs_tokW[p, cb, nkb, f] = hs[nk=nkb*128+p, c=cb*128+f]
            hs_tokW = bwork.tile([P, KB, KB, P], BF16, tag="hstok")
            nc.sync.dma_start(out=hs_tokW[:], in_=hsT[:], transpose=True)
            hsn = bwork.tile([P, KB, C], BF16, tag="hsn")
            st_s = _stats_via_reduce(
                nc, bwork, hs_tokW[:].rearrange("p c n f -> p n c f"),
                epss, "srn")
            for n in range(KB):
                nc.vector.tensor_scalar(
                    out=hsn[:, n, :].rearrange("p (c f) -> p c f", c=KB),
                    in0=hs_tokW[:, :, n, :],
                    scalar1=st_s[:, 2, n:n + 1], scalar2=st_s[:, 5, n:n + 1],
                    op0=OP.subtract, op1=OP.mult)
            # hsnW[p, nkb, cb, f] = hsn_val[nk=nkb*128+f, c=cb*128+p]
            hsnW = bwork.tile([P, KB, KB, P], BF16, tag="hsnT")
            nc.sync.dma_start(out=hsnW[:], in_=hsn[:], transpose=True)

            # k^T [c, nk]
            kps = mm_ps.tile([P, KB, NK], F32, tag="mmc", name="psk")
            for cb in range(KB):
                for kb in range(KB):
                    nc.tensor.matmul(
                        kps[:, cb, :], kw_sb[:, kb, cb * P:(cb + 1) * P],
                        hsnW[:, :, kb, :],
                        start=(kb == 0), stop=(kb == KB - 1))
            if nz["kb"]:
                for cb in range(KB):
                    nc.vector.tensor_scalar(
                        out=kT[:, cb, :], in0=kps[:, cb, :],
                        scalar1=bias_sb["kb"][:, cb:cb + 1],
                        scalar2=None, op0=OP.add)
            else:
                nc.vector.tensor_copy(out=kT[:], in_=kps[:])
            # v (token-major, (1-a) folded) -> fp8 planes; va = a/(1-a)*v
            vps = mm_ps.tile([P, KB, C], F32, tag="mmc", name="psv")
            for nkb in range(KB):
                for kb in range(KB):
                    nc.tensor.matmul(
                        vps[:, nkb, :], hsnW[:, nkb, kb, :],
                        vw_sb[:, kb, :],
                        start=(kb == 0), stop=(kb == KB - 1))
            if nz["vb"]:
                for nkb in range(KB):
                    nc.vector.tensor_add(
                        out=v8[:, nkb, :], in0=vps[:, nkb, :],
                        in1=bias_sb["vb"][:])
            else:
                nc.vector.tensor_copy(out=v8[:], in_=vps[:])

            # fp8 DR matmuls require dst partition 0: build zero-padded
            # block-diagonal stationary tiles (per-head 32-col blocks) so
            # each head's DR matmul writes the full [128,512] PSUM tile.
            # One strided write per tile: the hh dim advances by 32 cols
            # INSIDE the block as well as by one block, so its AP step is
            # block_stride + 32 (APs are linear in the indices).
            def _diag_ap(t, ghk_dims):
                base = t if isinstance(t, bass.AP) else t[:]
                ap = [list(p) for p in base.ap]
                # dims: [p, hg?, hh, kb, c32] - bump the hh step by 32
                hh_dim = len(ap) - 3
                ap[hh_dim][0] += 32
                return bass.AP(base.tensor, base.offset,
                               [tuple(p) for p in ap])

            nc.gpsimd.memset(v8bd[:], 0.0)
            nc.gpsimd.memset(onesbd[:], 0.0)
            nc.gpsimd.memset(_diag_ap(onesbd[:, :, :, 0:32], None), 1.0)
            vsrc = v8[:].rearrange("p k (g h c) -> p g h k c", g=KB, h=4)
            nc.vector.tensor_copy(
                out=_diag_ap(v8bd[:, :, :, :, 0:32], None), in_=vsrc)

        # ========== phase C: attention ===============================
        with ExitStack() as pctx:
            cwork = pctx.enter_context(tc.tile_pool(name="cwork", bufs=1))
            ot8full = cwork.tile([P, KB, N], FP8, tag="ot8full",
                                 name="ot8full")
            opsb = cwork.tile([P, KB, N], BF16, tag="opsb",
                              name="opsb")

            # op pre-phase: opsb[c, t] = (alpha * pos @ v)^T. One posD
            # buffer reused across both quads, so quad 1's matmuls repeat
            # quad 0's stationary sequence with only DMA/ACT instructions
            # between - the dedup pass keeps just 8 ldweights total.
            with ExitStack() as octx:
                opool = octx.enter_context(tc.tile_pool(name="opd",
                                                        bufs=1))
                op_ps = octx.enter_context(
                    tc.tile_pool(name="opps", bufs=1, space="PSUM"))
                posD = opool.tile([P, KB, KB, 4, 2048], FP8, tag="posD",
                                  name="posD")
                for quad in range(2):
                    qsl = slice(quad * 2048, (quad + 1) * 2048)
                    nc.sync.dma_start(posD[:], pos_d[:, :, :, :, qsl])
                    opps = op_ps.tile([P, KB, 4, 512], F32, tag="opps",
                                      name="opps")
                    hgs = range(KB) if quad == 0 else reversed(range(KB))
                    for hg in hgs:
                        hhs = range(4) if quad == 0 else reversed(range(4))
                        for hh in hhs:
                            first, last = ((0, 3) if quad == 0 else (3, 0))
                            for tq in range(4):
                                nc.tensor.matmul(
                                    opps[:, hg, tq, :], v8bd[:, hg, hh, :, :],
                                    posD[:, hg, :, hh,
                                         tq * 512:(tq + 1) * 512],
                                    start=(hh == first), stop=(hh == last),
                                    perf_mode=DR)
                    nc.scalar.copy(
                        out=opsb[:, :, qsl].rearrange(
                            "p k (q f) -> p k q f", q=4),
                        in_=opps[:])

            # --- scores (strip-resident stationaries), softmax exp, then
            # G / attn@v merged across both head-groups per quad ----------
            for quad in range(2):
                qsl = slice(quad * 2048, (quad + 1) * 2048)
                es8q = cwork.tile([P, KB, 4, KB, 2048], FP8, tag="es8q",
                                  name="es8q")
                for hg in range(KB):
                    with ExitStack() as sctx:
                        s_ps = sctx.enter_context(tc.tile_pool(
                            name="sps", bufs=1, space="PSUM"))
                        for nkb in range(KB):
                            sps = s_ps.tile([P, 4, 2, 512], F32,
                                            tag="sps", name="sps")
                            for half in range(2):
                                for hh in range(4):
                                    hb = slice(32 * hh, 32 * (hh + 1))
                                    for tg in range(2):
                                        t0 = (quad * 4 + half * 2
                                              + tg) * 512
                                        nc.tensor.matmul(
                                            sps[:, hh, tg, :],
                                            kT[hb, hg,
                                               nkb * P:(nkb + 1) * P],
                                            qT[hb, hg, t0:t0 + 512],
                                            start=True, stop=True,
                                            tile_position=(32 * hh, 0))
                                hsl = slice(half * 1024, (half + 1) * 1024)
                                nc.scalar.activation(
                                    es8q[:, hg, :, nkb, hsl]
                                    .rearrange("p h (g f) -> p h g f",
                                               g=2),
                                    sps[:], AF.Exp, scale=scale)
                with ExitStack() as goctx:
                  gwork = goctx.enter_context(
                      tc.tile_pool(name="gwork", bufs=1))
                  with ExitStack() as gctx:
                    g_ps = gctx.enter_context(tc.tile_pool(
                        name="gps", bufs=1, space="PSUM"))
                    gps = g_ps.tile([P, KB, 4, 512], F32, tag="gps",
                                    name="gps")
                    for hh in range(4):
                        for hg in range(KB):
                            for tq in range(4):
                                nc.tensor.matmul(
                                    gps[:, hg, tq, :], onesbd[:, hh, :, :],
                                    es8q[:, hg, hh, :,
                                         tq * 512:(tq + 1) * 512],
                                    start=(hh == 0), stop=(hh == 3),
                                    perf_mode=DR)
                    gsb = gwork.tile([P, KB, 4, 512], F32, tag="gsb",
                                     name="gsb")
                    nc.vector.reciprocal(gsb[:], gps[:])
                  with ExitStack() as gctx:
                    oe_ps = gctx.enter_context(tc.tile_pool(
                        name="oeps", bufs=1, space="PSUM"))
                    oeps = oe_ps.tile([P, KB, 4, 512], F32, tag="oeps",
                                      name="oeps")
                    for hh in range(4):
                        for hg in range(KB):
                            for tq in range(4):
                                nc.tensor.matmul(
                                    oeps[:, hg, tq, :], v8bd[:, hg, hh, :, :],
                                    es8q[:, hg, hh, :,
                                         tq * 512:(tq + 1) * 512],
                                    start=(hh == 0), stop=(hh == 3),
                                    perf_mode=DR)
                    tmpc = gwork.tile([P, KB, 4, 512], BF16, tag="tmpc",
                                      name="tmpc")
                    nc.vector.tensor_tensor(
                        out=tmpc[:], in0=oeps[:], in1=gsb[:], op=OP.mult)
                    nc.vector.scalar_tensor_tensor(
                        out=ot8full[:, :, qsl].rearrange(
                            "p k (q f) -> p k q f", q=4),
                        in0=opsb[:, :, qsl].rearrange(
                            "p k (q f) -> p k q f", q=4),
                        scalar=av_s, in1=tmpc[:],
                        op0=OP.mult, op1=OP.add)

            # --- proj, computed transposed (pT[c,t]), then one batched
            # xbar transpose per c-block and one residual add -------------
            with ExitStack() as pjctx:
                pjpool = pjctx.enter_context(tc.tile_pool(name="pjp",
                                                          bufs=1))
                pj_ps = pjctx.enter_context(tc.tile_pool(
                    name="pjps", bufs=1, space="PSUM"))
                for cb in range(KB):
                    pjps = pj_ps.tile([P, 8, 512], F32, tag="pjps",
                                      name="pjps")
                    for t8 in range(8):
                        nc.tensor.matmul(
                            pjps[:, t8, :], pjw_sb[:, :, cb * P:(cb + 1) * P],
                            ot8full[:, :, t8 * 512:(t8 + 1) * 512],
                            start=True, stop=True, perf_mode=DR)
                    pjsb = pjpool.tile([P, 8, 512], BF16, tag="pjsb",
                                       name="pjsb")
                    nc.scalar.copy(out=pjsb[:], in_=pjps[:])
                    pjT = pjpool.tile([P, TT, P], BF16, tag="pjT",
                                      name="pjT")
                    nc.sync.dma_start(out=pjT[:], in_=pjsb[:],
                                      transpose=True)
                    nc.vector.tensor_tensor(
                        out=x2[:, :, cb * P:(cb + 1) * P],
                        in0=xfull[:, :, cb * P:(cb + 1) * P],
                        in1=pjT[:], op=OP.add)
            if nz["pjb"]:
                for tt in range(TT):
                    nc.vector.tensor_add(
                        out=x2[:, tt, 0:C], in0=x2[:, tt, 0:C],
                        in1=bias_sb["pjb"][:])

        # ---- ln2 + h2^T (batched stats, one transpose, one fp8 cast) ----
        with ExitStack() as l2ctx:
            l2pool = l2ctx.enter_context(tc.tile_pool(name="l2p", bufs=1))
            st2 = _stats_via_reduce(nc, l2pool, x2[:, :, 0:C], eps1, "ln2")
            h2c = l2pool.tile([P, TT, C], BF16, tag="h2c", name="h2c")
            for t in range(TT):
                nc.vector.tensor_scalar(
                    out=h2c[:, t, :], in0=x2[:, t, 0:C],
                    scalar1=st2[:, 2, t:t + 1], scalar2=st2[:, 5, t:t + 1],
                    op0=OP.subtract, op1=OP.mult)
            h2W = l2pool.tile([P, TT, KB, P], BF16, tag="h2W", name="h2W")
            nc.sync.dma_start(out=h2W[:], in_=h2c[:], transpose=True)
            nc.scalar.copy(
                out=h2T8[:].rearrange("p k (t f) -> p k t f", t=TT),
                in_=h2W[:].rearrange("p t k f -> p k t f"))

        # ========== phase D: MLP =====================================
        with ExitStack() as pctx:
            m2cp = pctx.enter_context(tc.tile_pool(name="m2c", bufs=1))
            accp = pctx.enter_context(tc.tile_pool(name="accd", bufs=2))

            # two rotating padded layouts; borders zeroed once
            mpads = [m2cp.tile([P, NPAD], BF16, tag=f"mpad{j}",
                               name=f"mpad{j}")
                     for j in range(2)]
            for mp in mpads:
                nc.gpsimd.memset(mp[:], 0.0)

            m2pairs = []
            with ExitStack() as f1ctx:
                mm_ps = f1ctx.enter_context(
                    tc.tile_pool(name="mmD", bufs=1, space="PSUM"))
                for mb in range(MB):
                    mpad = mpads[mb % 2]
                    vp = mpad[:].rearrange("p (r c) -> p r c", c=PADW)
                    # one stationary, 8 matmuls into all 8 PSUM banks
                    ps = mm_ps.tile([P, 8, 512], F32, tag="mmd", name="psf1")
                    for nt in range(8):
                        nc.tensor.matmul(
                            ps[:, nt, :],
                            f1w_sb[:, :, mb * P:(mb + 1) * P],
                            h2T8[:, :, nt * 512:(nt + 1) * 512],
                            start=True, stop=True, perf_mode=DR)
                    dst = vp[:, 1:65, 1:65].rearrange(
                        "p (a r) c -> p a r c", a=8)
                    src = ps.rearrange("p k (r c) -> p k r c", c=HW)
                    if nz["f1b"]:
                        nc.scalar.tensor_scalar(
                            out=dst, in0=src,
                            scalar1=bias_sb["f1b"][:, mb:mb + 1],
                            scalar2=None, op0=OP.add)
                    else:
                        nc.scalar.copy(out=dst, in_=src)
                    # depthwise 3x3 conv: 9 accumulations (DVE + Pool split)
                    acc = accp.tile([P, HW * HW], BF16, tag="acc",
                                    name=f"acc{mb}")
                    av = acc[:].rearrange("p (r c) -> p r c", c=HW)
                    nc.vector.tensor_scalar(
                        out=av[:], in0=vp[:, 0:HW, 0:HW],
                        scalar1=dww_sb[:, mb, 0:1], scalar2=None, op0=OP.mult)
                    for tap in range(1, 9):
                        dy, dx = tap // 3, tap % 3
                        nc.vector.scalar_tensor_tensor(
                            out=av[:], in0=vp[:, dy:dy + HW, dx:dx + HW],
                            scalar=dww_sb[:, mb, tap:tap + 1], in1=av[:],
                            op0=OP.mult, op1=OP.add)
                    if mb % 2 == 0:
                        m2pair = m2cp.tile([P, 2, N], FP8, tag=f"m2c{mb // 2}",
                                           name=f"m2pair{mb}")
                        m2pairs.append(m2pair)
                    nc.scalar.activation(
                        m2pair[:, mb % 2, :].rearrange("p (r c) -> p r c",
                                                       c=HW),
                        av[:], AF.Gelu, bias=dwb_sb[:, mb:mb + 1])
            # fc2 transposed (yT[c,t], fp8 DR over all 4 hid pair-groups),
            # stationary-outer: per c-block, 4 ldweights feed 32 matmuls
            # across all 8 token groups (8 PSUM banks), then one eviction,
            # one batched transpose and one residual add.
            with ExitStack() as f2ctx:
                ypool = f2ctx.enter_context(tc.tile_pool(name="yp", bufs=1))
                f2_ps = f2ctx.enter_context(
                    tc.tile_pool(name="f2ps", bufs=1, space="PSUM"))
                for cb in range(KB):
                    yps = f2_ps.tile([P, 8, 512], F32, tag="fps", name="yps")
                    for g in range(4):
                        for ttg in range(8):
                            nc.tensor.matmul(
                                yps[:, ttg, :],
                                f2w_sb[:, g, :, cb * P:(cb + 1) * P],
                                m2pairs[g][:, :, ttg * 512:(ttg + 1) * 512],
                                start=(g == 0), stop=(g == 3), perf_mode=DR)
                    yt = ypool.tile([P, 8, 512], BF16, tag="yt", name="yt")
                    nc.scalar.copy(out=yt[:], in_=yps[:])
                    y4 = ypool.tile([P, TT, P], BF16, tag="y4", name="y4")
                    nc.sync.dma_start(out=y4[:], in_=yt[:], transpose=True)
                    nc.vector.tensor_tensor(
                        out=x2[:, :, cb * P:(cb + 1) * P],
                        in0=x2[:, :, cb * P:(cb + 1) * P],
                        in1=y4[:], op=OP.add)
            if nz["f2b"]:
                for tt in range(TT):
                    nc.vector.tensor_add(
                        out=x2[:, tt, 0:C], in0=x2[:, tt, 0:C],
                        in1=bias_sb["f2b"][:])

            nc.sync.dma_start(
                out_d.rearrange("(q p) c -> p q c", p=P),
                x2[:, :, 0:C])

    return nc


def _prep_pos(pos_b):
    """pos[H, N, NK] f32 -> [p, hg, nkb, hh, N] fp8 (transposed per head)."""
    arr = pos_b.reshape(KB, 4, N, KB, P).transpose(4, 0, 3, 1, 2)
    return _f8(arr)


def _run(inputs, trace=False):
    a = float(np.asarray(inputs["alpha"]).reshape(-1)[0])
    w = _prep_weights(inputs, a)
    nz = {nm: bool(np.any(w[nm])) for nm in
          ("qb", "srb", "kb", "vb", "pjb", "f1b", "f2b")}
    nc = _build_program(a, nz)
    _move_matmul_waits(nc)
    _dedup_ldweights(nc)
    _merge_waits(nc)
    _split_drain_waits(nc)

    x = np.asarray(inputs["x"], np.float32)
    pos = np.asarray(inputs["pos_2D"], np.float32)
    shared = {k: v for k, v in w.items()
              if k in ("qw8", "srw8", "kw", "vw", "pjw8", "f1w8", "dww",
                       "dwb", "f2w8")}
    for nm in ("qb", "srb", "kb", "vb", "pjb", "f1b", "f2b"):
        if nz[nm]:
            shared[nm] = w[nm]
    in_maps = []
    for b in range(B):
        in_maps.append(dict(shared, x=np.ascontiguousarray(x[b]),
                            pos8=_prep_pos(pos[b])))
    res = run_bass_kernel_spmd(nc, in_maps, list(range(B)), trace=trace)
    out = np.stack([res.results[b]["out"] for b in range(B)]).astype(np.float32)
    return out, res


def kernel(**inputs) -> np.ndarray:
    out, _ = _run(inputs, trace=False)
    return out

